# revision 10
# baseline (speedup 1.0000x reference)
"""CLIP attention (B=8, N=1024, C=1024, H=16, Dh=64) on 8 Trainium2 cores.

Strategy: data-parallel over batch (one batch element per core, no
collectives). Host pre-transposes x and the weight matrices so the kernel
needs zero on-chip transposes:

  per-core DRAM inputs (all fp32):
    xt  [1024, 1024]       x[b]^T               (c, n)
    wqk [16, 8, 128, 128]  blocked W_qk^T:  wqk[dc, cc, p, d] = W[dc*128+d, cc*128+p]
    wv  [8, 128, 1024]     blocked W_v^T:   wv[cc, p, dv]     = W[2048+dv, cc*128+p]
    wo  [8, 128, 1024]     blocked W_out^T: wo[cc, p, e]      = Wout[e, cc*128+p]
    bqk [128, 16]          in_proj_bias[:2048] partition-major per d-chunk
    bv  [1, 1024]          in_proj_bias[2048:]
    bo  [1, 1024]          out_proj_bias
  output: y [1024, 1024]   (n, e)

On-chip dataflow (per core):
  qk^T [d, n]  = W_qk x^T   (+bias via ACT per-partition)      -> qkT sbuf fp32
  v    [n, dv] = x W_v^T    (+bias via K=1 ones-row matmul)    -> v sbuf bf16,
                                with a 64-wide ones block appended at cols 1024:1088
  per head h:
    S^T [k, q] = (k^T_h)^T q^T_h        (K=64 matmuls)
    expT       = exp(S^T / 8)           (ACT, bf16)
    PV: psum[0:64]  = unnormalized out^T_h
        psum[64:128] = softmax row-sums (from the ones block, strided lhsT AP)
    out^T_h = psum[0:64] / psum[64:128] (DVE divide)            -> outT sbuf
  y [n, e] = out^T^T W_out^T + bias     (K=1 ones-row matmul for bias)
"""
import sys

sys.path.insert(0, "/opt/trn_rl_repo")

import numpy as np
from contextlib import ExitStack

import bass_rust
import concourse.bass as bass
import concourse.tile as tile
import concourse.mybir as mybir
from concourse import bacc
from concourse.bass_utils import run_bass_kernel_spmd

F32 = mybir.dt.float32
F32R = mybir.dt.float32r
BF16 = mybir.dt.bfloat16
AF = mybir.ActivationFunctionType
ALU = mybir.AluOpType

P = 128
N = 1024
C = 1024
NH = 16
DH = 64

# matmul input dtype for the fp32-stored operands: F32R (full-rate PE) or F32
# (exact, 1/4 rate).  PV/exp dtype: BF16 or F32.
MM_DT = F32R
PV_BF16 = True

_CACHE = {}


def _skip_ap(ap, start, pairs):
    """AP with custom [step,count] dims appended after the partition dim."""
    c = ap.copy()
    part = c.ap.to_list()[0]
    c.ap = bass_rust.VecI64Pair([part] + pairs)
    c.offset = c.offset + start
    return c


def _mm(a):
    return a


def _emit(tc, t):
    nc = tc.nc
    pv_dt = BF16 if PV_BF16 else F32
    with ExitStack() as ctx:
        const = ctx.enter_context(tc.tile_pool(name="const", bufs=1))
        persist = ctx.enter_context(tc.tile_pool(name="persist", bufs=1))
        qkTp = ctx.enter_context(tc.tile_pool(name="qkTp", bufs=1))

        bqk_sb = const.tile([P, 16], F32, tag="bqk")
        bv_sb = const.tile([1, C], F32R, tag="bv")
        bo_sb = const.tile([1, C], F32R, tag="bo")
        ones_sb = const.tile([1, P], F32R, tag="ones")
        ones_f32 = const.tile([1, P], F32, tag="ones32")
        nc.sync.dma_start(bqk_sb[:], t["bqk"][:])
        nc.sync.dma_start(bv_sb[:], t["bv"][:])
        nc.sync.dma_start(bo_sb[:], t["bo"][:])
        nc.any.memset(ones_f32[:], 1.0)
        nc.vector.tensor_copy(ones_sb[:], ones_f32[:])

        qkT = qkTp.tile([P, 16, N], F32R, tag="qkT")
        # v layout: [k-part, k-chunk, head, 64 v-cols | 64 ones-cols]; the ones
        # columns make the PV matmul emit softmax row-sums in psum rows 64:128.
        v_sb = persist.tile([P, 8, NH, P], pv_dt, tag="v")
        outT = persist.tile([P, 8, N], F32R, tag="outT")
        nc.any.memset(v_sb[:, :, :, 64:P], 1.0)

        with tc.tile_pool(name="xtp", bufs=8) as xtp, \
             tc.tile_pool(name="wqkp", bufs=2) as wqkp, \
             tc.tile_pool(name="wvp", bufs=4) as wvp:
            # ---------------- phase B-qk: qk^T = W_qk @ x^T + b ----------------
            with tc.tile_pool(name="qkps", bufs=2, space="PSUM") as qkps:
                def load_wqk(dc):
                    w = wqkp.tile([P, 8, P], F32R, tag="wqk", name=f"wt_{dc}")
                    src_ap = t["wqk"][dc].rearrange("c p d -> p c d")
                    nc.sync.dma_start(w[:, 0:4, :], src_ap[:, 0:4, :])
                    nc.sync.dma_start(w[:, 4:8, :], src_ap[:, 4:8, :])
                    return w

                wts = {0: load_wqk(0)}
                xt_c = []
                for cc in range(8):
                    xtc = xtp.tile([P, N], F32R, tag="xt", name=f"xt_{cc}")
                    nc.sync.dma_start(xtc[:], t["xt"][cc * P:(cc + 1) * P, :])
                    xt_c.append(xtc)
                for dc in range(16):
                    if dc not in wts:
                        wts[dc] = load_wqk(dc)
                    wt = wts.pop(dc)
                    qp = qkps.tile([P, N], F32, tag="qkp")
                    for nb in range(2):
                        sl = slice(nb * 512, (nb + 1) * 512)
                        for cc in range(8):
                            nc.tensor.matmul(
                                qp[:, sl], wt[:, cc, :], xt_c[cc][:, sl],
                                start=(cc == 0), stop=(cc == 7))
                    nc.scalar.activation(qkT[:, dc, :], qp[:], AF.Identity,
                                         bias=bqk_sb[:, dc:dc + 1], scale=1.0)

            # ---------------- phase B-v: v = x @ W_v^T + b_v ----------------
            with tc.tile_pool(name="vps", bufs=4, space="PSUM") as vps:
                for grp in range(2):
                    vp = [vps.tile([P, C], F32, tag="vp", name=f"vp_{grp}_{j}")
                          for j in range(4)]
                    for j in range(4):
                        for db in range(2):
                            sl = slice(db * 512, (db + 1) * 512)
                            nc.tensor.matmul(vp[j][:, sl], ones_sb[:],
                                             bv_sb[:, sl], start=True, stop=False)
                    for cc in range(8):
                        wvt = wvp.tile([P, C], F32R, tag="wv",
                                       name=f"wv_{grp}_{cc}")
                        nc.sync.dma_start(wvt[:, 0:512], t["wv"][cc][:, 0:512])
                        nc.sync.dma_start(wvt[:, 512:C], t["wv"][cc][:, 512:C])
                        for j in range(4):
                            nsl = slice((grp * 4 + j) * P, (grp * 4 + j + 1) * P)
                            for db in range(2):
                                sl = slice(db * 512, (db + 1) * 512)
                                nc.tensor.matmul(vp[j][:, sl],
                                                 xt_c[cc][:, nsl],
                                                 wvt[:, sl],
                                                 start=False, stop=(cc == 7))
                    for j in range(4):
                        nc.vector.tensor_copy(v_sb[:, grp * 4 + j, :, 0:64],
                                              vp[j][:])

        # ---------------- phases C+D ----------------
        with tc.tile_pool(name="wop", bufs=1) as wop:
            # prefetch the out-projection weights during attention
            wo_sb = wop.tile([P, 8, C], F32R, tag="wo")
            for cc in range(8):
                nc.sync.dma_start(wo_sb[:, cc, 0:512], t["wo"][cc][:, 0:512])
                nc.sync.dma_start(wo_sb[:, cc, 512:C], t["wo"][cc][:, 512:C])

            # ------------- phase C: attention per head -------------
            with tc.tile_pool(name="eTp", bufs=2) as eTp, \
                 tc.tile_pool(name="sums", bufs=2) as sums_pool, \
                 tc.tile_pool(name="sps", bufs=2, space="PSUM") as sps, \
                 tc.tile_pool(name="pps", bufs=4, space="PSUM") as pps:

                def emit_pv(h, eT):
                    hb = (h % 2) * 64
                    for qb in range(2):
                        sl = slice(qb * 512, (qb + 1) * 512)
                        pp = pps.tile([P, 512], F32, tag="pp",
                                      name=f"pp_{h}_{qb}")
                        for kc in range(8):
                            nc.tensor.matmul(pp[:], v_sb[:, kc, h, :],
                                             eT[:, kc, sl],
                                             start=(kc == 0), stop=(kc == 7))
                        # no divide ALU on DVE: 1/s = exp(-ln(s)) on ACT (one
                        # table set holds ln+exp), ln runs in-place in PSUM.
                        nc.scalar.activation(pp[64:128, :], pp[64:128, :], AF.Ln)
                        rec = sums_pool.tile([64, 512], F32, tag="rec",
                                             name=f"rec_{h}_{qb}")
                        nc.scalar.activation(rec[:], pp[64:128, :], AF.Exp,
                                             scale=-1.0)
                        nc.vector.tensor_tensor(outT[hb:hb + 64, h // 2, sl],
                                                pp[0:64, :], rec[:], ALU.mult)

                prev = None
                for h in range(NH):
                    hb = (h % 2) * 64
                    dcq = h // 2
                    dck = 8 + h // 2
                    eT = eTp.tile([P, 8, N], pv_dt, tag="eT", name=f"eT_{h}")
                    if prev is not None:
                        emit_pv(*prev)
                    for kc in range(8):
                        sp = sps.tile([P, N], F32, tag="sp", name=f"sp_{h}_{kc}")
                        for qb in range(2):
                            sl = slice(qb * 512, (qb + 1) * 512)
                            nc.tensor.matmul(
                                sp[:, sl],
                                qkT[hb:hb + 64, dck, kc * P:(kc + 1) * P],
                                qkT[hb:hb + 64, dcq, sl],
                                start=True, stop=True)
                        nc.scalar.activation(eT[:, kc, :], sp[:], AF.Exp,
                                             scale=0.125)
                    prev = (h, eT)
                emit_pv(*prev)

            # ------------- phase D: y = out @ W_out^T + b_o -------------
            with tc.tile_pool(name="yps", bufs=4, space="PSUM") as yps, \
                 tc.tile_pool(name="ysb", bufs=3) as ysbp:
                for grp in range(2):
                    yp = [yps.tile([P, C], F32, tag="yp", name=f"yp_{grp}_{j}")
                          for j in range(4)]
                    for j in range(4):
                        for eb in range(2):
                            sl = slice(eb * 512, (eb + 1) * 512)
                            nc.tensor.matmul(yp[j][:, sl], ones_sb[:],
                                             bo_sb[:, sl],
                                             start=True, stop=False)
                    for cc in range(8):
                        for j in range(4):
                            nsl = slice((grp * 4 + j) * P, (grp * 4 + j + 1) * P)
                            for eb in range(2):
                                sl = slice(eb * 512, (eb + 1) * 512)
                                nc.tensor.matmul(yp[j][:, sl],
                                                 outT[:, cc, nsl],
                                                 wo_sb[:, cc, sl],
                                                 start=False, stop=(cc == 7))
                    for j in range(4):
                        ysb = ysbp.tile([P, C], F32, tag="y")
                        nc.scalar.copy(ysb[:], yp[j][:])
                        nc.sync.dma_start(
                            t["y"][(grp * 4 + j) * P:(grp * 4 + j + 1) * P, :],
                            ysb[:])


def build_nc():
    if "nc" in _CACHE:
        return _CACHE["nc"]
    nc = bacc.Bacc("TRN2", target_bir_lowering=False, debug=False, num_devices=8)
    t = {
        "xt": nc.dram_tensor("xt", [C, N], F32R, kind="ExternalInput"),
        "wqk": nc.dram_tensor("wqk", [16, 8, P, P], F32R, kind="ExternalInput"),
        "wv": nc.dram_tensor("wv", [8, P, C], F32R, kind="ExternalInput"),
        "wo": nc.dram_tensor("wo", [8, P, C], F32R, kind="ExternalInput"),
        "bqk": nc.dram_tensor("bqk", [P, 16], F32, kind="ExternalInput"),
        "bv": nc.dram_tensor("bv", [1, C], F32R, kind="ExternalInput"),
        "bo": nc.dram_tensor("bo", [1, C], F32R, kind="ExternalInput"),
        "y": nc.dram_tensor("y", [N, C], F32, kind="ExternalOutput"),
    }
    with tile.TileContext(nc) as tc:
        _emit(tc, t)
    nc.compile()
    _CACHE["nc"] = nc
    return nc


def host_prep(x, in_proj_weight, in_proj_bias, out_proj_weight, out_proj_bias):
    """Host-side sharding + pre-transposes. Returns per-core in_maps."""
    x = np.asarray(x, dtype=np.float32)
    w_in = np.asarray(in_proj_weight, dtype=np.float32)
    b_in = np.asarray(in_proj_bias, dtype=np.float32)
    w_out = np.asarray(out_proj_weight, dtype=np.float32)
    b_out = np.asarray(out_proj_bias, dtype=np.float32)

    wqkT = np.ascontiguousarray(w_in[:2 * C].T)          # [1024, 2048]
    wqk = np.ascontiguousarray(
        wqkT.reshape(8, P, 16, P).transpose(2, 0, 1, 3))  # [16, 8, 128, 128]
    wv = np.ascontiguousarray(w_in[2 * C:].T).reshape(8, P, C)
    wo = np.ascontiguousarray(w_out.T).reshape(8, P, C)
    bqk = np.ascontiguousarray(b_in[:2 * C].reshape(16, P).T)  # [128, 16]
    bv = np.ascontiguousarray(b_in[2 * C:].reshape(1, C))
    bo = np.ascontiguousarray(b_out.reshape(1, C))

    shared = {"wqk": wqk, "wv": wv, "wo": wo, "bqk": bqk, "bv": bv, "bo": bo}
    in_maps = []
    for b in range(x.shape[0]):
        m = dict(shared)
        m["xt"] = np.ascontiguousarray(x[b].T)
        in_maps.append(m)
    return in_maps


def kernel(x, in_proj_weight, in_proj_bias, out_proj_weight, out_proj_bias,
           **run_kwargs):
    in_maps = host_prep(x, in_proj_weight, in_proj_bias, out_proj_weight,
                        out_proj_bias)
    nc = build_nc()
    res = run_bass_kernel_spmd(nc, in_maps, core_ids=list(range(len(in_maps))),
                               **run_kwargs)
    out = np.stack([r["y"] for r in res.results], axis=0)
    if run_kwargs:
        kernel.last_results = res
    return out


# revision 11
# speedup vs baseline: 1.0088x; 1.0088x over previous
"""CLIP attention (B=8, N=1024, C=1024, H=16, Dh=64) on 8 Trainium2 cores.

Strategy: data-parallel over batch (one batch element per core, no
collectives). Host pre-transposes x and the weight matrices so the kernel
needs zero on-chip transposes:

  per-core DRAM inputs (all fp32):
    xt  [1024, 1024]       x[b]^T               (c, n)
    wqk [16, 8, 128, 128]  blocked W_qk^T:  wqk[dc, cc, p, d] = W[dc*128+d, cc*128+p]
    wv  [8, 128, 1024]     blocked W_v^T:   wv[cc, p, dv]     = W[2048+dv, cc*128+p]
    wo  [8, 128, 1024]     blocked W_out^T: wo[cc, p, e]      = Wout[e, cc*128+p]
    bqk [128, 16]          in_proj_bias[:2048] partition-major per d-chunk
    bv  [1, 1024]          in_proj_bias[2048:]
    bo  [1, 1024]          out_proj_bias
  output: y [1024, 1024]   (n, e)

On-chip dataflow (per core):
  qk^T [d, n]  = W_qk x^T   (+bias via ACT per-partition)      -> qkT sbuf fp32
  v    [n, dv] = x W_v^T    (+bias via K=1 ones-row matmul)    -> v sbuf bf16,
                                with a 64-wide ones block appended at cols 1024:1088
  per head h:
    S^T [k, q] = (k^T_h)^T q^T_h        (K=64 matmuls)
    expT       = exp(S^T / 8)           (ACT, bf16)
    PV: psum[0:64]  = unnormalized out^T_h
        psum[64:128] = softmax row-sums (from the ones block, strided lhsT AP)
    out^T_h = psum[0:64] / psum[64:128] (DVE divide)            -> outT sbuf
  y [n, e] = out^T^T W_out^T + bias     (K=1 ones-row matmul for bias)
"""
import sys

sys.path.insert(0, "/opt/trn_rl_repo")

import numpy as np
from contextlib import ExitStack

import bass_rust
import concourse.bass as bass
import concourse.tile as tile
import concourse.mybir as mybir
from concourse import bacc
from concourse.bass_utils import run_bass_kernel_spmd

F32 = mybir.dt.float32
F32R = mybir.dt.float32r
BF16 = mybir.dt.bfloat16
AF = mybir.ActivationFunctionType
ALU = mybir.AluOpType

P = 128
N = 1024
C = 1024
NH = 16
DH = 64

# matmul input dtype for the fp32-stored operands: F32R (full-rate PE) or F32
# (exact, 1/4 rate).  PV/exp dtype: BF16 or F32.
MM_DT = F32R
PV_BF16 = True

_CACHE = {}


def _skip_ap(ap, start, pairs):
    """AP with custom [step,count] dims appended after the partition dim."""
    c = ap.copy()
    part = c.ap.to_list()[0]
    c.ap = bass_rust.VecI64Pair([part] + pairs)
    c.offset = c.offset + start
    return c


def _mm(a):
    return a


def _emit(tc, t):
    nc = tc.nc
    pv_dt = BF16 if PV_BF16 else F32
    with ExitStack() as ctx:
        const = ctx.enter_context(tc.tile_pool(name="const", bufs=1))
        persist = ctx.enter_context(tc.tile_pool(name="persist", bufs=1))
        qkTp = ctx.enter_context(tc.tile_pool(name="qkTp", bufs=1))

        bqk_sb = const.tile([P, 16], F32, tag="bqk")
        bv_sb = const.tile([1, C], F32R, tag="bv")
        bo_sb = const.tile([1, C], F32R, tag="bo")
        ones_sb = const.tile([1, P], F32R, tag="ones")
        ones_f32 = const.tile([1, P], F32, tag="ones32")
        nc.sync.dma_start(bqk_sb[:], t["bqk"][:])
        nc.sync.dma_start(bv_sb[:], t["bv"][:])
        nc.sync.dma_start(bo_sb[:], t["bo"][:])
        nc.any.memset(ones_f32[:], 1.0)
        nc.vector.tensor_copy(ones_sb[:], ones_f32[:])

        qkT = qkTp.tile([P, 16, N], F32R, tag="qkT")
        # v layout: [k-part, k-chunk, head, 64 v-cols | 32 ones-cols]; the ones
        # columns make the PV matmul emit softmax row-sums in psum rows 64:96.
        v_sb = persist.tile([P, 8, NH, 96], pv_dt, tag="v")
        outT = persist.tile([P, 8, N], F32R, tag="outT")
        nc.any.memset(v_sb[:, :, :, 64:96], 1.0)

        with tc.tile_pool(name="xtp", bufs=8) as xtp, \
             tc.tile_pool(name="wqkp", bufs=2) as wqkp, \
             tc.tile_pool(name="wvp", bufs=4) as wvp:
            # ---------------- phase B-qk: qk^T = W_qk @ x^T + b ----------------
            with tc.tile_pool(name="qkps", bufs=2, space="PSUM") as qkps:
                def load_wqk(dc):
                    w = wqkp.tile([P, 8, P], F32R, tag="wqk", name=f"wt_{dc}")
                    src_ap = t["wqk"][dc].rearrange("c p d -> p c d")
                    nc.sync.dma_start(w[:, 0:4, :], src_ap[:, 0:4, :])
                    nc.sync.dma_start(w[:, 4:8, :], src_ap[:, 4:8, :])
                    return w

                wts = {0: load_wqk(0)}
                xt_c = []
                for cc in range(8):
                    xtc = xtp.tile([P, N], F32R, tag="xt", name=f"xt_{cc}")
                    nc.sync.dma_start(xtc[:], t["xt"][cc * P:(cc + 1) * P, :])
                    xt_c.append(xtc)
                for dc in range(16):
                    if dc not in wts:
                        wts[dc] = load_wqk(dc)
                    wt = wts.pop(dc)
                    qp = qkps.tile([P, N], F32, tag="qkp")
                    for nb in range(2):
                        sl = slice(nb * 512, (nb + 1) * 512)
                        for cc in range(8):
                            nc.tensor.matmul(
                                qp[:, sl], wt[:, cc, :], xt_c[cc][:, sl],
                                start=(cc == 0), stop=(cc == 7))
                    nc.scalar.activation(qkT[:, dc, :], qp[:], AF.Identity,
                                         bias=bqk_sb[:, dc:dc + 1], scale=1.0)

            # ---------------- phase B-v: v = x @ W_v^T + b_v ----------------
            with tc.tile_pool(name="vps", bufs=4, space="PSUM") as vps:
                for grp in range(2):
                    vp = [vps.tile([P, C], F32, tag="vp", name=f"vp_{grp}_{j}")
                          for j in range(4)]
                    for j in range(4):
                        for db in range(2):
                            sl = slice(db * 512, (db + 1) * 512)
                            nc.tensor.matmul(vp[j][:, sl], ones_sb[:],
                                             bv_sb[:, sl], start=True, stop=False)
                    for cc in range(8):
                        wvt = wvp.tile([P, C], F32R, tag="wv",
                                       name=f"wv_{grp}_{cc}")
                        nc.sync.dma_start(wvt[:, 0:512], t["wv"][cc][:, 0:512])
                        nc.sync.dma_start(wvt[:, 512:C], t["wv"][cc][:, 512:C])
                        for j in range(4):
                            nsl = slice((grp * 4 + j) * P, (grp * 4 + j + 1) * P)
                            for db in range(2):
                                sl = slice(db * 512, (db + 1) * 512)
                                nc.tensor.matmul(vp[j][:, sl],
                                                 xt_c[cc][:, nsl],
                                                 wvt[:, sl],
                                                 start=False, stop=(cc == 7))
                    for j in range(4):
                        nc.vector.tensor_copy(v_sb[:, grp * 4 + j, :, 0:64],
                                              vp[j][:])

        # ---------------- phases C+D ----------------
        with tc.tile_pool(name="wop", bufs=1) as wop:
            # prefetch half the out-projection weights during attention
            wo_lo = wop.tile([P, 8, 512], F32R, tag="wo_lo")
            for cc in range(8):
                nc.sync.dma_start(wo_lo[:, cc, :], t["wo"][cc][:, 0:512])

            # ------------- phase C: attention per head -------------
            with tc.tile_pool(name="eTp", bufs=3) as eTp, \
                 tc.tile_pool(name="sums", bufs=2) as sums_pool, \
                 tc.tile_pool(name="sps", bufs=2, space="PSUM") as sps, \
                 tc.tile_pool(name="pps", bufs=4, space="PSUM") as pps:

                def emit_pv(h, eT):
                    hb = (h % 2) * 64
                    for qb in range(2):
                        sl = slice(qb * 512, (qb + 1) * 512)
                        pp = pps.tile([96, 512], F32, tag="pp",
                                      name=f"pp_{h}_{qb}")
                        for kc in range(8):
                            nc.tensor.matmul(pp[:], v_sb[:, kc, h, :],
                                             eT[:, kc, sl],
                                             start=(kc == 0), stop=(kc == 7))
                        # no divide ALU on DVE: 1/s = exp(-ln(s)) on ACT (one
                        # table set holds ln+exp), ln runs in-place in PSUM.
                        nc.scalar.activation(pp[64:96, :], pp[64:96, :], AF.Ln)
                        rec = sums_pool.tile([32, 512], F32, tag="rec",
                                             name=f"rec_{h}_{qb}")
                        nc.scalar.activation(rec[:], pp[64:96, :], AF.Exp,
                                             scale=-1.0)
                        nc.vector.tensor_tensor(outT[hb:hb + 32, h // 2, sl],
                                                pp[0:32, :], rec[:], ALU.mult)
                        nc.vector.tensor_tensor(outT[hb + 32:hb + 64, h // 2, sl],
                                                pp[32:64, :], rec[:], ALU.mult)

                pend = []
                for h in range(NH):
                    hb = (h % 2) * 64
                    dcq = h // 2
                    dck = 8 + h // 2
                    eT = eTp.tile([P, 8, N], pv_dt, tag="eT", name=f"eT_{h}")
                    if len(pend) >= 2:
                        emit_pv(*pend.pop(0))
                    for kc in range(8):
                        sp = sps.tile([P, N], F32, tag="sp", name=f"sp_{h}_{kc}")
                        for qb in range(2):
                            sl = slice(qb * 512, (qb + 1) * 512)
                            nc.tensor.matmul(
                                sp[:, sl],
                                qkT[hb:hb + 64, dck, kc * P:(kc + 1) * P],
                                qkT[hb:hb + 64, dcq, sl],
                                start=True, stop=True)
                        nc.scalar.activation(eT[:, kc, :], sp[:], AF.Exp,
                                             scale=0.125)
                    pend.append((h, eT))
                for pv in pend:
                    emit_pv(*pv)

            # ------------- phase D: y = out @ W_out^T + b_o -------------
            with tc.tile_pool(name="yps", bufs=8, space="PSUM") as yps, \
                 tc.tile_pool(name="wohp", bufs=1) as wohp, \
                 tc.tile_pool(name="ysb", bufs=3) as ysbp:
                wo_hi = wohp.tile([P, 8, 512], F32R, tag="wo_hi")
                for cc in range(8):
                    nc.sync.dma_start(wo_hi[:, cc, :], t["wo"][cc][:, 512:C])
                wo_half = [wo_lo, wo_hi]
                for grp in range(2):
                    yp = [[yps.tile([P, 512], F32, tag="yp",
                                    name=f"yp_{grp}_{j}_{eb}")
                           for eb in range(2)] for j in range(4)]
                    for eb in range(2):
                        sl = slice(eb * 512, (eb + 1) * 512)
                        for j in range(4):
                            nc.tensor.matmul(yp[j][eb][:], ones_sb[:],
                                             bo_sb[:, sl],
                                             start=True, stop=False)
                        for cc in range(8):
                            for j in range(4):
                                nsl = slice((grp * 4 + j) * P,
                                            (grp * 4 + j + 1) * P)
                                nc.tensor.matmul(yp[j][eb][:],
                                                 outT[:, cc, nsl],
                                                 wo_half[eb][:, cc, :],
                                                 start=False, stop=(cc == 7))
                    for j in range(4):
                        ysb = ysbp.tile([P, C], F32, tag="y")
                        nc.scalar.copy(ysb[:, 0:512], yp[j][0][:])
                        nc.scalar.copy(ysb[:, 512:C], yp[j][1][:])
                        nc.sync.dma_start(
                            t["y"][(grp * 4 + j) * P:(grp * 4 + j + 1) * P, :],
                            ysb[:])


def build_nc():
    if "nc" in _CACHE:
        return _CACHE["nc"]
    nc = bacc.Bacc("TRN2", target_bir_lowering=False, debug=False, num_devices=8)
    t = {
        "xt": nc.dram_tensor("xt", [C, N], F32R, kind="ExternalInput"),
        "wqk": nc.dram_tensor("wqk", [16, 8, P, P], F32R, kind="ExternalInput"),
        "wv": nc.dram_tensor("wv", [8, P, C], F32R, kind="ExternalInput"),
        "wo": nc.dram_tensor("wo", [8, P, C], F32R, kind="ExternalInput"),
        "bqk": nc.dram_tensor("bqk", [P, 16], F32, kind="ExternalInput"),
        "bv": nc.dram_tensor("bv", [1, C], F32R, kind="ExternalInput"),
        "bo": nc.dram_tensor("bo", [1, C], F32R, kind="ExternalInput"),
        "y": nc.dram_tensor("y", [N, C], F32, kind="ExternalOutput"),
    }
    with tile.TileContext(nc) as tc:
        _emit(tc, t)
    nc.compile()
    _CACHE["nc"] = nc
    return nc


def host_prep(x, in_proj_weight, in_proj_bias, out_proj_weight, out_proj_bias):
    """Host-side sharding + pre-transposes. Returns per-core in_maps."""
    x = np.asarray(x, dtype=np.float32)
    w_in = np.asarray(in_proj_weight, dtype=np.float32)
    b_in = np.asarray(in_proj_bias, dtype=np.float32)
    w_out = np.asarray(out_proj_weight, dtype=np.float32)
    b_out = np.asarray(out_proj_bias, dtype=np.float32)

    wqkT = np.ascontiguousarray(w_in[:2 * C].T)          # [1024, 2048]
    wqk = np.ascontiguousarray(
        wqkT.reshape(8, P, 16, P).transpose(2, 0, 1, 3))  # [16, 8, 128, 128]
    wv = np.ascontiguousarray(w_in[2 * C:].T).reshape(8, P, C)
    wo = np.ascontiguousarray(w_out.T).reshape(8, P, C)
    bqk = np.ascontiguousarray(b_in[:2 * C].reshape(16, P).T)  # [128, 16]
    bv = np.ascontiguousarray(b_in[2 * C:].reshape(1, C))
    bo = np.ascontiguousarray(b_out.reshape(1, C))

    shared = {"wqk": wqk, "wv": wv, "wo": wo, "bqk": bqk, "bv": bv, "bo": bo}
    in_maps = []
    for b in range(x.shape[0]):
        m = dict(shared)
        m["xt"] = np.ascontiguousarray(x[b].T)
        in_maps.append(m)
    return in_maps


def kernel(x, in_proj_weight, in_proj_bias, out_proj_weight, out_proj_bias,
           **run_kwargs):
    in_maps = host_prep(x, in_proj_weight, in_proj_bias, out_proj_weight,
                        out_proj_bias)
    nc = build_nc()
    res = run_bass_kernel_spmd(nc, in_maps, core_ids=list(range(len(in_maps))),
                               **run_kwargs)
    out = np.stack([r["y"] for r in res.results], axis=0)
    if run_kwargs:
        kernel.last_results = res
    return out


# revision 12
# speedup vs baseline: 1.3131x; 1.3017x over previous
"""CLIP attention (B=8, N=1024, C=1024, H=16, Dh=64) on 8 Trainium2 cores.

Strategy: data-parallel over batch (one batch element per core, no
collectives). Host pre-transposes x and the weight matrices so the kernel
needs zero on-chip transposes:

  per-core DRAM inputs (all fp32):
    xt  [1024, 1024]       x[b]^T               (c, n)
    wqk [16, 8, 128, 128]  blocked W_qk^T:  wqk[dc, cc, p, d] = W[dc*128+d, cc*128+p]
    wv  [8, 128, 1024]     blocked W_v^T:   wv[cc, p, dv]     = W[2048+dv, cc*128+p]
    wo  [8, 128, 1024]     blocked W_out^T: wo[cc, p, e]      = Wout[e, cc*128+p]
    bqk [128, 16]          in_proj_bias[:2048] partition-major per d-chunk
    bv  [1, 1024]          in_proj_bias[2048:]
    bo  [1, 1024]          out_proj_bias
  output: y [1024, 1024]   (n, e)

On-chip dataflow (per core):
  qk^T [d, n]  = W_qk x^T   (+bias via ACT per-partition)      -> qkT sbuf fp32
  v    [n, dv] = x W_v^T    (+bias via K=1 ones-row matmul)    -> v sbuf bf16,
                                with a 64-wide ones block appended at cols 1024:1088
  per head h:
    S^T [k, q] = (k^T_h)^T q^T_h        (K=64 matmuls)
    expT       = exp(S^T / 8)           (ACT, bf16)
    PV: psum[0:64]  = unnormalized out^T_h
        psum[64:128] = softmax row-sums (from the ones block, strided lhsT AP)
    out^T_h = psum[0:64] / psum[64:128] (DVE divide)            -> outT sbuf
  y [n, e] = out^T^T W_out^T + bias     (K=1 ones-row matmul for bias)
"""
import sys

sys.path.insert(0, "/opt/trn_rl_repo")

import functools
import numpy as np
from contextlib import ExitStack

import bass_rust
import concourse.bass as bass
import concourse.tile as tile
import concourse.mybir as mybir
from concourse import bacc
from concourse.bass_utils import run_bass_kernel_spmd

F32 = mybir.dt.float32
F32R = mybir.dt.float32r
BF16 = mybir.dt.bfloat16
AF = mybir.ActivationFunctionType
ALU = mybir.AluOpType

P = 128
N = 1024
C = 1024
NH = 16
DH = 64

# matmul input dtype for the fp32-stored operands: F32R (full-rate PE) or F32
# (exact, 1/4 rate).  PV/exp dtype: BF16 or F32.
MM_DT = F32R
PV_BF16 = True

_CACHE = {}


def _skip_ap(ap, start, pairs):
    """AP with custom [step,count] dims appended after the partition dim."""
    c = ap.copy()
    part = c.ap.to_list()[0]
    c.ap = bass_rust.VecI64Pair([part] + pairs)
    c.offset = c.offset + start
    return c


def _mm(a):
    return a


def _patch_act_tables():
    """Pin exp/ln/identity/copy to the one ACT table set containing them all
    (natural_log_exp_and_others) so the kernel never reloads table sets.
    Without this, walrus picks exp_and_others for Exp and the natural-log set
    for Ln, thrashing the 1.3us ACT_TABLE_LOAD twice per attention head."""
    import concourse.hw_specs as hw_specs
    if getattr(hw_specs, "_act_tables_patched", False):
        return
    orig = hw_specs.get_activation_tables
    keep = "natural_log_exp_and_others"
    pin = {AF.Exp, AF.Ln, AF.Identity, AF.Copy}

    @functools.cache
    def patched(arch):
        tabs = orig(arch)
        return {
            name: set(fns) if name == keep else set(fns) - pin
            for name, fns in tabs.items()
        }

    hw_specs.get_activation_tables = patched
    bacc.get_activation_tables = patched
    hw_specs._act_tables_patched = True


def _emit(tc, t):
    nc = tc.nc
    pv_dt = BF16 if PV_BF16 else F32
    with ExitStack() as ctx:
        const = ctx.enter_context(tc.tile_pool(name="const", bufs=1))
        persist = ctx.enter_context(tc.tile_pool(name="persist", bufs=1))

        bqk_sb = const.tile([P, 16], F32, tag="bqk")
        bv_sb = const.tile([1, C], F32R, tag="bv")
        bo_sb = const.tile([1, C], F32R, tag="bo")
        ones_sb = const.tile([1, P], F32R, tag="ones")
        ones_f32 = const.tile([1, P], F32, tag="ones32")
        nc.sync.dma_start(bqk_sb[:], t["bqk"][:])
        nc.sync.dma_start(bv_sb[:], t["bv"][:])
        nc.sync.dma_start(bo_sb[:], t["bo"][:])
        nc.any.memset(ones_f32[:], 1.0)
        nc.vector.tensor_copy(ones_sb[:], ones_f32[:])

        # v layout: [k-part, k-chunk, head, 64 v-cols | 32 ones-cols]; the ones
        # columns make the PV matmul emit softmax row-sums in psum rows 64:96.
        v_sb = persist.tile([P, 8, NH, 96], pv_dt, tag="v")
        outT = persist.tile([P, 8, N], F32R, tag="outT")
        nc.any.memset(v_sb[:, :, :, 64:96], 1.0)

        with tc.tile_pool(name="xtp", bufs=8) as xtp:
            xt_c = []
            for cc in range(8):
                xtc = xtp.tile([P, N], F32R, tag="xt", name=f"xt_{cc}")
                nc.sync.dma_start(xtc[:], t["xt"][cc * P:(cc + 1) * P, :])
                xt_c.append(xtc)

            # ---------------- phase B-v: v = x @ W_v^T + b_v ----------------
            with tc.tile_pool(name="wvp", bufs=4) as wvp, \
                 tc.tile_pool(name="vps", bufs=4, space="PSUM") as vps:
                for grp in range(2):
                    vp = [vps.tile([P, C], F32, tag="vp", name=f"vp_{grp}_{j}")
                          for j in range(4)]
                    for j in range(4):
                        for db in range(2):
                            sl = slice(db * 512, (db + 1) * 512)
                            nc.tensor.matmul(vp[j][:, sl], ones_sb[:],
                                             bv_sb[:, sl], start=True, stop=False)
                    for cc in range(8):
                        wvt = wvp.tile([P, C], F32R, tag="wv",
                                       name=f"wv_{grp}_{cc}")
                        nc.sync.dma_start(wvt[:, 0:512], t["wv"][cc][:, 0:512])
                        nc.sync.dma_start(wvt[:, 512:C], t["wv"][cc][:, 512:C])
                        for j in range(4):
                            nsl = slice((grp * 4 + j) * P, (grp * 4 + j + 1) * P)
                            for db in range(2):
                                sl = slice(db * 512, (db + 1) * 512)
                                nc.tensor.matmul(vp[j][:, sl],
                                                 xt_c[cc][:, nsl],
                                                 wvt[:, sl],
                                                 start=False, stop=(cc == 7))
                    for j in range(4):
                        nc.vector.tensor_copy(v_sb[:, grp * 4 + j, :, 0:64],
                                              vp[j][:])

            # ------- interleaved B-qk (qk projection) + attention -------
            # Per head-pair j: project qkT chunks (j, 8+j), then S/exp for
            # heads 2j, 2j+1 with PV lagging one head. Keeps the PE stream
            # dense (projection + S + PV) while ACT streams the exps, so the
            # HAM clock gate stays open.
            with tc.tile_pool(name="qkTp", bufs=16) as qkTp, \
                 tc.tile_pool(name="wqkp", bufs=2) as wqkp, \
                 tc.tile_pool(name="eTp", bufs=2) as eTp, \
                 tc.tile_pool(name="sums", bufs=2) as sums_pool, \
                 tc.tile_pool(name="qkps", bufs=2, space="PSUM") as qkps, \
                 tc.tile_pool(name="sps", bufs=2, space="PSUM") as sps, \
                 tc.tile_pool(name="pps", bufs=2, space="PSUM") as pps:

                qkT = {}

                def emit_bqk(dc):
                    wt = wqkp.tile([P, 8, P], F32R, tag="wqk", name=f"wt_{dc}")
                    srcp = t["wqk"][dc].rearrange("c p d -> p c d")
                    nc.sync.dma_start(wt[:, 0:4, :], srcp[:, 0:4, :])
                    nc.sync.dma_start(wt[:, 4:8, :], srcp[:, 4:8, :])
                    qkT[dc] = qkTp.tile([P, N], F32R, tag="qkT",
                                        name=f"qkT_{dc}")
                    bias_bc = bqk_sb[:, dc:dc + 1].to_broadcast((P, 512))
                    for nb in range(2):
                        sl = slice(nb * 512, (nb + 1) * 512)
                        qp = qkps.tile([P, 512], F32, tag="qkp",
                                       name=f"qkp_{dc}_{nb}")
                        for cc in range(8):
                            nc.tensor.matmul(
                                qp[:], wt[:, cc, :], xt_c[cc][:, sl],
                                start=(cc == 0), stop=(cc == 7))
                        nc.vector.tensor_tensor(qkT[dc][:, sl], qp[:],
                                                bias_bc, ALU.add)

                def emit_s(h):
                    hb = (h % 2) * 64
                    qT = qkT[h // 2]
                    kT = qkT[8 + h // 2]
                    eT = eTp.tile([P, 8, N], pv_dt, tag="eT", name=f"eT_{h}")
                    for kc in range(8):
                        sp = sps.tile([P, N], F32, tag="sp", name=f"sp_{h}_{kc}")
                        for qb in range(2):
                            sl = slice(qb * 512, (qb + 1) * 512)
                            nc.tensor.matmul(
                                sp[:, sl],
                                kT[hb:hb + 64, kc * P:(kc + 1) * P],
                                qT[hb:hb + 64, sl],
                                start=True, stop=True)
                        nc.scalar.activation(eT[:, kc, :], sp[:], AF.Exp,
                                             scale=0.125)
                    return eT

                def emit_pv(h, eT):
                    hb = (h % 2) * 64
                    for qb in range(2):
                        sl = slice(qb * 512, (qb + 1) * 512)
                        pp = pps.tile([96, 512], F32, tag="pp",
                                      name=f"pp_{h}_{qb}")
                        for kc in range(8):
                            nc.tensor.matmul(pp[:], v_sb[:, kc, h, :],
                                             eT[:, kc, sl],
                                             start=(kc == 0), stop=(kc == 7))
                        # 1/s = exp(-ln(s)); ln in-place in PSUM. Both live in
                        # one ACT table set (see _patch_act_tables).
                        nc.scalar.activation(pp[64:96, :], pp[64:96, :], AF.Ln)
                        rec = sums_pool.tile([32, 512], F32, tag="rec",
                                             name=f"rec_{h}_{qb}")
                        nc.scalar.activation(rec[:], pp[64:96, :], AF.Exp,
                                             scale=-1.0)
                        nc.vector.tensor_tensor(outT[hb:hb + 32, h // 2, sl],
                                                pp[0:32, :], rec[:], ALU.mult)
                        nc.vector.tensor_tensor(outT[hb + 32:hb + 64, h // 2, sl],
                                                pp[32:64, :], rec[:], ALU.mult)

                pend = None
                for j in range(8):
                    emit_bqk(j)
                    emit_bqk(8 + j)
                    eT0 = emit_s(2 * j)
                    if pend is not None:
                        emit_pv(*pend)
                    eT1 = emit_s(2 * j + 1)
                    emit_pv(2 * j, eT0)
                    pend = (2 * j + 1, eT1)
                emit_pv(*pend)

        # ------------- phase D: y = out @ W_out^T + b_o -------------
        with tc.tile_pool(name="wop", bufs=1) as wop, \
             tc.tile_pool(name="yps", bufs=8, space="PSUM") as yps, \
             tc.tile_pool(name="ysb", bufs=3) as ysbp:
            wo_sb = wop.tile([P, 8, C], F32R, tag="wo")
            for cc in range(8):
                nc.sync.dma_start(wo_sb[:, cc, 0:512], t["wo"][cc][:, 0:512])
                nc.sync.dma_start(wo_sb[:, cc, 512:C], t["wo"][cc][:, 512:C])
            for grp in range(2):
                yp = [[yps.tile([P, 512], F32, tag="yp",
                                name=f"yp_{grp}_{j}_{eb}")
                       for eb in range(2)] for j in range(4)]
                for eb in range(2):
                    sl = slice(eb * 512, (eb + 1) * 512)
                    for j in range(4):
                        nc.tensor.matmul(yp[j][eb][:], ones_sb[:],
                                         bo_sb[:, sl],
                                         start=True, stop=False)
                    for cc in range(8):
                        for j in range(4):
                            nsl = slice((grp * 4 + j) * P,
                                        (grp * 4 + j + 1) * P)
                            nc.tensor.matmul(yp[j][eb][:],
                                             outT[:, cc, nsl],
                                             wo_sb[:, cc, sl],
                                             start=False, stop=(cc == 7))
                for j in range(4):
                    ysb = ysbp.tile([P, C], F32, tag="y")
                    nc.scalar.copy(ysb[:, 0:512], yp[j][0][:])
                    nc.scalar.copy(ysb[:, 512:C], yp[j][1][:])
                    nc.sync.dma_start(
                        t["y"][(grp * 4 + j) * P:(grp * 4 + j + 1) * P, :],
                        ysb[:])


def build_nc():
    if "nc" in _CACHE:
        return _CACHE["nc"]
    _patch_act_tables()
    nc = bacc.Bacc("TRN2", target_bir_lowering=False, debug=False, num_devices=8)
    t = {
        "xt": nc.dram_tensor("xt", [C, N], F32R, kind="ExternalInput"),
        "wqk": nc.dram_tensor("wqk", [16, 8, P, P], F32R, kind="ExternalInput"),
        "wv": nc.dram_tensor("wv", [8, P, C], F32R, kind="ExternalInput"),
        "wo": nc.dram_tensor("wo", [8, P, C], F32R, kind="ExternalInput"),
        "bqk": nc.dram_tensor("bqk", [P, 16], F32, kind="ExternalInput"),
        "bv": nc.dram_tensor("bv", [1, C], F32R, kind="ExternalInput"),
        "bo": nc.dram_tensor("bo", [1, C], F32R, kind="ExternalInput"),
        "y": nc.dram_tensor("y", [N, C], F32, kind="ExternalOutput"),
    }
    with tile.TileContext(nc) as tc:
        _emit(tc, t)
    nc.compile()
    _CACHE["nc"] = nc
    return nc


def host_prep(x, in_proj_weight, in_proj_bias, out_proj_weight, out_proj_bias):
    """Host-side sharding + pre-transposes. Returns per-core in_maps."""
    x = np.asarray(x, dtype=np.float32)
    w_in = np.asarray(in_proj_weight, dtype=np.float32)
    b_in = np.asarray(in_proj_bias, dtype=np.float32)
    w_out = np.asarray(out_proj_weight, dtype=np.float32)
    b_out = np.asarray(out_proj_bias, dtype=np.float32)

    wqkT = np.ascontiguousarray(w_in[:2 * C].T)          # [1024, 2048]
    wqk = np.ascontiguousarray(
        wqkT.reshape(8, P, 16, P).transpose(2, 0, 1, 3))  # [16, 8, 128, 128]
    wv = np.ascontiguousarray(w_in[2 * C:].T).reshape(8, P, C)
    wo = np.ascontiguousarray(w_out.T).reshape(8, P, C)
    bqk = np.ascontiguousarray(b_in[:2 * C].reshape(16, P).T)  # [128, 16]
    bv = np.ascontiguousarray(b_in[2 * C:].reshape(1, C))
    bo = np.ascontiguousarray(b_out.reshape(1, C))

    shared = {"wqk": wqk, "wv": wv, "wo": wo, "bqk": bqk, "bv": bv, "bo": bo}
    in_maps = []
    for b in range(x.shape[0]):
        m = dict(shared)
        m["xt"] = np.ascontiguousarray(x[b].T)
        in_maps.append(m)
    return in_maps


def kernel(x, in_proj_weight, in_proj_bias, out_proj_weight, out_proj_bias,
           **run_kwargs):
    in_maps = host_prep(x, in_proj_weight, in_proj_bias, out_proj_weight,
                        out_proj_bias)
    nc = build_nc()
    res = run_bass_kernel_spmd(nc, in_maps, core_ids=list(range(len(in_maps))),
                               **run_kwargs)
    out = np.stack([r["y"] for r in res.results], axis=0)
    if run_kwargs:
        kernel.last_results = res
    return out
